# revision 1
# baseline (speedup 1.0000x reference)
"""Trainium2 Bass kernel for block-causal (chunked) multi-head attention.

Computes, for x:[2,2048,1024], Wqkv:[3072,1024], Wout:[1024,1024]:
    qkv = x @ Wqkv.T ; per-head scaled scores; block-causal mask
    (causal OR same 64-chunk == full attention to all chunks <= own chunk);
    softmax; out = attn @ v ; y = out @ Wout.T

Sharding over 8 NeuronCores: data-parallel over batch (2) x tensor-parallel
over heads (16 heads -> 4 per core).  Each core projects q/k/v for its 4
heads, runs attention, and computes a partial output projection against its
256 columns of Wout; the host sums the 4 partials per batch element.

On-chip layout avoids all transposes: the host hands each core
  xT     [1024, 2048]  (x[b] transposed)
  wqkT   [1024, 512]   (Wqkv rows for its 4 heads' q,k -> transposed)
  wvT    [1024, 256]   (v rows transposed)
  woutT  [256, 1024]   (Wout columns for its head-slice, transposed)
Scores are computed transposed (S^T[tk, tq]) so that the attention matmul
P^T -> (attn @ V) needs no transposes, and the softmax denominator comes
for free from a ones-column appended to V.  The block-causal mask is
realized structurally: masked-out key blocks are simply never computed, and
the diagonal blocks use rectangular sub-views (chunk granularity 64).

Engines execute their instruction streams in order, so the emission is a
software pipeline over the 4 query tiles: the TensorE stream for the
(ScalarE-paced) attention of tile t is interleaved with "filler" matmul
chains -- the q/k/v projections of tile t+1 and the output projection of
tile t-1 -- keeping the PE busy through every exp dependency stall.
"""

import sys

if "/opt/trn_rl_repo" not in sys.path:
    sys.path.insert(0, "/opt/trn_rl_repo")

from collections import deque

import numpy as np

import concourse.bass as bass  # noqa: F401  (registers types)
import concourse.mybir as mybir
import concourse.tile as tile
from concourse import bacc
from concourse.bass_utils import run_bass_kernel_spmd

F32 = mybir.dt.float32
F32R = mybir.dt.float32r
EXP = mybir.ActivationFunctionType.Exp

B = 2
T = 2048
DIM = 1024
N_HEADS = 16
HD = 64
CHUNK = 64
H_PER_CORE = 4  # 16 heads / (8 cores / 2 batches)
QT = 512  # query tile (free dim of S^T matmuls)
KB = 128  # key block (contraction block of AV matmuls)
N_QT = T // QT  # 4
N_KB = T // KB  # 16
N_DIMB = DIM // 128  # 8 contraction blocks for the projections
SCALE = 1.0 / np.sqrt(HD)

SPLIT_Y = False
_CACHED_NC = None


def _emit(nc, tc, xT, wqkT, wvT, woT, y):
    po = tc.tile_pool  # shorthand

    with (
        po(name="persist", bufs=1) as pp,
        po(name="s_ps", bufs=2, space="PSUM") as sps,  # [128,1024] score slots
        po(name="mm_ps", bufs=2, space="PSUM") as mmps,  # [128,512] proj/y slots
        po(name="ot_ps", bufs=2, space="PSUM") as otps,  # [65,512] outT slots
        po(name="pbuf", bufs=4) as ppool,  # exp(S^T) tiles
        po(name="osbuf", bufs=2) as ospool,  # assembled normalized outT
        po(name="scbuf", bufs=2) as scpool,  # normalize scratch
        po(name="rbuf", bufs=2) as rpool,  # reciprocal denominators
        po(name="ybuf", bufs=3) as ypool,
    ):
        # ---- persistent SBUF tensors (chunked to keep deps fine-grained) ----
        xt = [
            [pp.tile([128, QT], F32R, tag=f"xt{k}_{c}", name=f"xt{k}_{c}") for c in range(N_QT)]
            for k in range(N_DIMB)
        ]
        wqk = [pp.tile([128, 512], F32R, tag=f"wqk{k}", name=f"wqk{k}") for k in range(N_DIMB)]
        wv = [pp.tile([128, 256], F32R, tag=f"wv{k}", name=f"wv{k}") for k in range(N_DIMB)]
        wo = [pp.tile([128, DIM], F32R, tag=f"wo{d}", name=f"wo{d}") for d in range(2)]
        # q/k head-dim-major: partition block hp holds heads (2hp, 2hp+1)
        qt = [
            [pp.tile([128, QT], F32R, tag=f"qt{i}_{c}", name=f"qt{i}_{c}") for c in range(N_QT)]
            for i in range(2)
        ]
        kt = [
            [pp.tile([128, QT], F32R, tag=f"kt{i}_{c}", name=f"kt{i}_{c}") for c in range(N_QT)]
            for i in range(2)
        ]
        # v (token-major) + ones column, per key block: [128, 4 heads, 65]
        vh = [
            pp.tile([128, H_PER_CORE, 2 * HD], F32R, tag=f"vh{b}", name=f"vh{b}")
            for b in range(N_KB)
        ]
        # ones row for the K=1 denominator-broadcast matmuls (row 64 used)
        ones = pp.tile([128, 64], F32R, tag="ones", name="ones")
        nc.vector.memset(ones[:].bitcast(F32), 1.0)

        # ---- input DMAs: kb-major so the kb=0..7 chains fill in order; the
        # xT columns arrive chunk-by-chunk so tile 0's projections start early
        for kb in range(N_DIMB):
            nc.sync.dma_start(wqk[kb][:], wqkT[kb * 128 : (kb + 1) * 128, :])
            nc.sync.dma_start(xt[kb][0][:], xT[kb * 128 : (kb + 1) * 128, 0:QT])
        for kb in range(N_DIMB):
            nc.sync.dma_start(wv[kb][:], wvT[kb * 128 : (kb + 1) * 128, :])
        for ct in range(1, N_QT):
            cs = slice(ct * QT, (ct + 1) * QT)
            for kb in range(N_DIMB):
                nc.sync.dma_start(xt[kb][ct][:], xT[kb * 128 : (kb + 1) * 128, cs])
        for db in range(2):
            nc.sync.dma_start(wo[db][:], woT[db * 128 : (db + 1) * 128, :])

        def qk_chain(tt, ob):  # ob 0,1 -> q pair blocks; 2,3 -> k pair blocks
            ps = mmps.tile([128, 512], F32, tag="mm512", name=f"qk_ps{tt}_{ob}")
            for kb in range(N_DIMB):
                nc.tensor.matmul(
                    ps[:],
                    wqk[kb][:, ob * 128 : (ob + 1) * 128],
                    xt[kb][tt][:],
                    start=(kb == 0),
                    stop=(kb == N_DIMB - 1),
                )
            dest = (qt if ob < 2 else kt)[ob % 2][tt]
            nc.vector.tensor_copy(dest[:], ps[:])

        def v_chain(tb):
            ps = mmps.tile([128, 256], F32, tag="mm512", name=f"v_ps{tb}")
            for kb in range(N_DIMB):
                nc.tensor.matmul(
                    ps[:],
                    xt[kb][tb // 4][:, (tb % 4) * KB : (tb % 4 + 1) * KB],
                    wv[kb][:],
                    start=(kb == 0),
                    stop=(kb == N_DIMB - 1),
                )
            nc.vector.tensor_copy(vh[tb][:, :, 0:HD], ps[:])
            nc.vector.memset(vh[tb][:, :, HD : 2 * HD].bitcast(F32), 1.0)

        def proj_pieces(tt):
            for ob in range(4):
                yield lambda ob=ob: qk_chain(tt, ob)
            for tb in range(4 * tt, 4 * tt + 4):
                yield lambda tb=tb: v_chain(tb)

        def y_pieces_split(tt, os_pair):
            """Output projection split per head-pair half: the os_pair[0]
            halves (a) can run as fillers inside attend(tt) right after pair
            0's normalize; the os_pair[1] halves (b) accumulate via DVE adds
            once pair 1 lands."""
            ysbs = {}

            def get_ysb(t4):
                if t4 not in ysbs:
                    ysbs[t4] = ypool.tile(
                        [128, DIM], F32, tag="ysb", name=f"ysb{tt}_{t4}"
                    )
                return ysbs[t4]

            a_pieces, b_pieces = [], []
            for t4 in range(4):
                trows = slice(t4 * 128, (t4 + 1) * 128)
                for jb in range(2):

                    def pa(t4=t4, jb=jb, trows=trows):
                        yps = mmps.tile(
                            [128, 512], F32, tag="mm512", name=f"ya{tt}_{t4}_{jb}"
                        )
                        nc.tensor.matmul(
                            yps[:],
                            os_pair[0][:, trows],
                            wo[0][:, jb * 512 : (jb + 1) * 512],
                            start=True,
                            stop=True,
                        )
                        nc.vector.tensor_copy(
                            get_ysb(t4)[:, jb * 512 : (jb + 1) * 512], yps[:]
                        )

                    def pb(t4=t4, jb=jb, trows=trows):
                        yps = mmps.tile(
                            [128, 512], F32, tag="mm512", name=f"yb{tt}_{t4}_{jb}"
                        )
                        nc.tensor.matmul(
                            yps[:],
                            os_pair[1][:, trows],
                            wo[1][:, jb * 512 : (jb + 1) * 512],
                            start=True,
                            stop=True,
                        )
                        ysb = get_ysb(t4)
                        dest = ysb[:, jb * 512 : (jb + 1) * 512]
                        nc.vector.tensor_add(dest, dest, yps[:])
                        if jb == 1:
                            nc.sync.dma_start(
                                y[tt * QT + t4 * 128 : tt * QT + (t4 + 1) * 128, :],
                                ysb[:],
                            )

                    a_pieces.append(pa)
                    b_pieces.append(pb)
            return a_pieces, b_pieces

        def attend(tt, os_pair, fillers, late=None):
            nb = 4 * (tt + 1)  # allowed key blocks for this query tile
            n_steps = 2 * nb
            step = 0
            done_fill = 0
            n_fill = len(fillers)
            late_q = deque()
            late_done = 0
            late_start = None

            def fill():
                nonlocal done_fill, late_done
                want = (step + 1) * n_fill // n_steps
                while done_fill < want and fillers:
                    fillers.popleft()()
                    done_fill += 1
                if late_start is not None and late_q:
                    lsteps = max(n_steps - late_start, 1)
                    lwant = (step - late_start + 1) * len_late // lsteps
                    while late_done < lwant and late_q:
                        late_q.popleft()()
                        late_done += 1

            for hp in range(2):  # head pair (2hp, 2hp+1)
                ot = [
                    otps.tile([128, QT], F32, tag="ot", name=f"ot{tt}_{hp}_{i}")
                    for i in range(2)
                ]

                def s_mm(b):
                    """S^T for key block b, both heads, into one 2-bank tile.
                    Also allocates the exp target and emits its mask memset
                    here, a block early, so the memset sits ahead of any
                    filler work in the in-order DVE stream."""
                    diag = b - 4 * tt
                    d = diag * 128 if diag >= 0 else 0
                    s = sps.tile([128, 2 * QT], F32, tag="s2", name=f"s{tt}_{hp}_{b}")
                    for i in range(2):
                        rows = slice(i * 64, i * 64 + 64)
                        nc.tensor.matmul(
                            s[:, i * QT + d : (i + 1) * QT],
                            kt[hp][b // 4][rows, (b % 4) * KB : (b % 4 + 1) * KB],
                            qt[hp][tt][rows, d:QT],
                            start=True,
                            stop=True,
                        )
                    p = ppool.tile([128, 2 * QT], F32R, tag="p", name=f"p{tt}_{hp}_{b}")
                    return s, p

                s_tiles = {0: s_mm(0)}
                for b in range(nb):
                    if b + 1 < nb:
                        s_tiles[b + 1] = s_mm(b + 1)
                    diag = b - 4 * tt
                    d = diag * 128 if diag >= 0 else 0
                    s, p = s_tiles.pop(b)
                    if diag < 0:
                        nc.scalar.activation(p[:], s[:], EXP, scale=SCALE)
                    else:
                        # one exp for both heads over cols >= d (all rows),
                        # then zero the masked corner (rows 64-127 of each
                        # head attend only cols >= d+64) AFTER the exp
                        s2 = s[:].rearrange("p (h c) -> p h c", h=2)
                        p2 = p[:].rearrange("p (h c) -> p h c", h=2)
                        nc.scalar.activation(
                            p2[:, :, d:QT], s2[:, :, d:QT], EXP, scale=SCALE
                        )
                        nc.vector.memset(
                            p2[64:128, :, d : d + 64].bitcast(F32), 0.0
                        )
                    for i in range(2):
                        nc.tensor.matmul(
                            ot[i][:, d:QT],
                            vh[b][:, 2 * hp + i, :],
                            p[:, i * QT + d : (i + 1) * QT],
                            start=(b == 0),
                            stop=(b == nb - 1),
                        )
                    fill()
                    step += 1

                # normalize: os_pair[hp][i*64:(i+1)*64] = ot[i][0:64] / ot[i][64]
                for i in range(2):
                    # denominator already replicated on partitions 64-127 by
                    # the 64 ones-columns in vhat; partition-shifted DVE
                    # reciprocal brings 1/denom to partitions 0-63
                    rb = rpool.tile([64, QT], F32, tag="rb", name=f"rb{tt}_{hp}_{i}")
                    nc.vector.reciprocal(rb[:], ot[i][64:128, :])
                    if i == 0:
                        # head at partitions 0-63: write os_pair directly
                        nc.vector.tensor_mul(
                            os_pair[hp][0:64, :], ot[i][0:64, :], rb[:]
                        )
                    else:
                        # head at partitions 64-127: partition-shifted DVE copy
                        sc = scpool.tile(
                            [64, QT], F32R, tag="sc", name=f"sc{tt}_{hp}_{i}"
                        )
                        nc.vector.tensor_mul(sc[:], ot[i][0:64, :], rb[:])
                        nc.vector.tensor_copy(os_pair[hp][64:128, :], sc[:])

                if hp == 0 and late:
                    late_q.extend(late)
                    late_start = step
                    len_late = len(late)

            while fillers:
                fillers.popleft()()
            while late_q:
                late_q.popleft()()

        def y_pieces_paired(tt, os_pair):
            pieces = []
            for t4 in range(4):
                trows = slice(t4 * 128, (t4 + 1) * 128)
                ysb = ypool.tile([128, DIM], F32, tag="ysb", name=f"ysb{tt}_{t4}")
                for jb in range(2):

                    def piece(t4=t4, jb=jb, ysb=ysb, trows=trows):
                        yps = mmps.tile(
                            [128, 512], F32, tag="mm512", name=f"y_ps{tt}_{t4}_{jb}"
                        )
                        for db in range(2):
                            nc.tensor.matmul(
                                yps[:],
                                os_pair[db][:, trows],
                                wo[db][:, jb * 512 : (jb + 1) * 512],
                                start=(db == 0),
                                stop=(db == 1),
                            )
                        nc.vector.tensor_copy(ysb[:, jb * 512 : (jb + 1) * 512], yps[:])
                        if jb == 1:
                            nc.sync.dma_start(
                                y[tt * QT + t4 * 128 : tt * QT + (t4 + 1) * 128, :],
                                ysb[:],
                            )

                    pieces.append(piece)
            return pieces

        # ---- the pipeline ----
        for piece in proj_pieces(0):
            piece()
        prev_b = None
        for tt in range(N_QT):
            os_pair = [
                ospool.tile([128, QT], F32R, tag=f"os{i}", name=f"os{i}_{tt}")
                for i in range(2)
            ]
            fillers = deque()
            a = deque(proj_pieces(tt + 1)) if tt + 1 < N_QT else deque()
            b = deque(prev_b) if prev_b is not None else deque()
            while a or b:
                if b:
                    fillers.append(b.popleft())
                if a:
                    fillers.append(a.popleft())
            split = SPLIT_Y and tt == N_QT - 1
            if split:
                a_pieces, b_pieces = y_pieces_split(tt, os_pair)
                attend(tt, os_pair, fillers, late=a_pieces)
                prev_b = b_pieces
            else:
                attend(tt, os_pair, fillers)
                prev_b = y_pieces_paired(tt, os_pair)
        for piece in prev_b:
            piece()


def build():
    global _CACHED_NC
    if _CACHED_NC is not None:
        return _CACHED_NC
    nc = bacc.Bacc(
        "TRN2", target_bir_lowering=False, debug=False, enable_asserts=False
    )
    xT = nc.dram_tensor("xT", [DIM, T], F32R, kind="ExternalInput").ap()
    wqkT = nc.dram_tensor("wqkT", [DIM, 512], F32R, kind="ExternalInput").ap()
    wvT = nc.dram_tensor("wvT", [DIM, 256], F32R, kind="ExternalInput").ap()
    woT = nc.dram_tensor("woutT", [256, DIM], F32R, kind="ExternalInput").ap()
    y = nc.dram_tensor("y", [T, DIM], F32, kind="ExternalOutput").ap()
    with tile.TileContext(nc) as tc:
        _emit(nc, tc, xT, wqkT, wvT, woT, y)
    nc.compile()
    _CACHED_NC = nc
    return nc


def make_in_maps(x, Wqkv, Wout):
    """Host-side sharding: core c = (batch c//4, head-group c%4)."""
    in_maps = []
    for c in range(8):
        b, hg = divmod(c, 4)
        hs = hg * H_PER_CORE
        r0, r1 = hs * HD, (hs + H_PER_CORE) * HD
        qrows = Wqkv[r0:r1]
        krows = Wqkv[DIM + r0 : DIM + r1]
        vrows = Wqkv[2 * DIM + r0 : 2 * DIM + r1]
        in_maps.append(
            {
                "xT": np.ascontiguousarray(x[b].T),
                "wqkT": np.ascontiguousarray(np.concatenate([qrows, krows], 0).T),
                "wvT": np.ascontiguousarray(vrows.T),
                "woutT": np.ascontiguousarray(Wout[:, r0:r1].T),
            }
        )
    return in_maps


def kernel(x, Wqkv, Wout):
    x = np.asarray(x, dtype=np.float32)
    Wqkv = np.asarray(Wqkv, dtype=np.float32)
    Wout = np.asarray(Wout, dtype=np.float32)
    nc = build()
    in_maps = make_in_maps(x, Wqkv, Wout)
    res = run_bass_kernel_spmd(nc, in_maps, core_ids=list(range(8)))
    out = np.zeros((B, T, DIM), np.float32)
    for c in range(8):
        out[c // 4] += res.results[c]["y"]
    return out



# revision 29
# speedup vs baseline: 1.1279x; 1.1279x over previous
"""Trainium2 Bass kernel for block-causal (chunked) multi-head attention.

Computes, for x:[2,2048,1024], Wqkv:[3072,1024], Wout:[1024,1024]:
    qkv = x @ Wqkv.T ; per-head scaled scores; block-causal mask
    (causal OR same 64-chunk == full attention to all chunks <= own chunk);
    softmax; out = attn @ v ; y = out @ Wout.T

Sharding over 8 NeuronCores: data-parallel over batch (2) x tensor-parallel
over heads (16 heads -> 4 per core).  Each core projects q/k/v for its 4
heads, runs attention, and computes a partial output projection against its
256 columns of Wout; the host sums the 4 partials per batch element.

v2 vs v1:
  * All matmul operands in bf16 (PE cost model: 1.0 cycles/row at any free
    size, vs fp32r's 4x penalty below 256) and all input DMA halved.
  * A tiny warm-up matmul at t~0 pins pe_busy_start so every real matmul
    after ~3us runs at the full 2.4 GHz p-state.
  * Tile-0 projections are emitted kb-major (all 8 psum chains accumulate in
    parallel) so the PE consumes each (wqk[kb], xt[kb]) DMA pair as it lands
    instead of stalling on a chain-major walk.
  * vh slot order [h0, h2 | h1, h3] with the ones-columns swapped to the
    low half for odd heads, so the softmax normalization needs no
    partition-shifted DVE copy: both heads' numerators land mul-aligned
    with their reciprocals.
  * y tiles are copied out of PSUM on alternating DVE/Act engines and
    DMA'd per 512-half; the last tile's normalize is split per 128-token
    slice so its y matmuls start before the full mul finishes.

Scores are computed transposed (S^T[tk, tq]) so that the attention matmul
needs no transposes, and the softmax denominator comes from ones-columns
interleaved in V.  The block-causal mask is realized structurally: masked
key blocks are never computed, and diagonal blocks use rectangular
sub-views (chunk granularity 64) with one small memset for the corner.
"""

import sys

if "/opt/trn_rl_repo" not in sys.path:
    sys.path.insert(0, "/opt/trn_rl_repo")

from collections import deque

import numpy as np

import concourse.bass as bass  # noqa: F401  (registers types)
import concourse.mybir as mybir
import concourse.tile as tile
from concourse import bacc
from concourse.bass_utils import run_bass_kernel_spmd

F32 = mybir.dt.float32
BF16 = mybir.dt.bfloat16
EXP = mybir.ActivationFunctionType.Exp
COPY = mybir.ActivationFunctionType.Copy

B = 2
T = 2048
DIM = 1024
N_HEADS = 16
HD = 64
CHUNK = 64
H_PER_CORE = 4  # 16 heads / (8 cores / 2 batches)
QT = 512  # query tile (free dim of S^T matmuls)
KB = 128  # key block (contraction block of AV matmuls)
N_QT = T // QT  # 4
N_KB = T // KB  # 16
N_DIMB = DIM // 128  # 8 contraction blocks for the projections
SCALE = 1.0 / np.sqrt(HD)

_CACHED_NC = None


def _emit(nc, tc, xT, wqkT, wvT, woT, y):
    po = tc.tile_pool  # shorthand

    with (
        po(name="persist", bufs=1) as pp,
        po(name="s_ps", bufs=2, space="PSUM") as sps,  # [128,1024] score slots
        po(name="mm_ps", bufs=2, space="PSUM") as mmps,  # [128,512] proj/y slots
        po(name="ot_ps", bufs=2, space="PSUM") as otps,  # [128,512] outT slots
        po(name="pbuf", bufs=6) as ppool,  # exp(S^T) tiles
        po(name="osbuf", bufs=2) as ospool,  # assembled normalized outT
        po(name="rbuf", bufs=2) as rpool,  # reciprocal denominators
        po(name="ybuf", bufs=6) as ypool,
    ):
        # ---- persistent SBUF tensors ----
        warm = pp.tile([128, 128], BF16, tag="warm", name="warm")
        # xt for column-tile 0: per-kb tiles (fine DMA deps feed the kb-major
        # projection); tiles 1-3: one [128, 8, 512] tile each, single DMA
        xt0 = [pp.tile([128, QT], BF16, tag=f"xt0_{k}", name=f"xt0_{k}") for k in range(N_DIMB)]
        xtc = [
            pp.tile([128, N_DIMB, QT], BF16, tag=f"xtc{c}", name=f"xtc{c}")
            for c in range(1, N_QT)
        ]
        wqk = [pp.tile([128, 512], BF16, tag=f"wqk{k}", name=f"wqk{k}") for k in range(N_DIMB)]
        wv = pp.tile([128, N_DIMB, 256], BF16, tag="wv", name="wv")
        wo = pp.tile([128, 2, DIM], BF16, tag="wo", name="wo")
        # q/k head-dim-major: partition block hp holds heads (2hp, 2hp+1)
        qt = [
            [pp.tile([128, QT], BF16, tag=f"qt{i}_{c}", name=f"qt{i}_{c}") for c in range(N_QT)]
            for i in range(2)
        ]
        kt = [
            [pp.tile([128, QT], BF16, tag=f"kt{i}_{c}", name=f"kt{i}_{c}") for c in range(N_QT)]
            for i in range(2)
        ]
        # v (token-major) + ones columns, per key block: [128, slot, 128]
        # slot order is [h0, h2, h1, h3]; even-slot layout [v | ones],
        # odd-head layout [ones | v] (so AV's denominator lands where the
        # normalize mul wants it).  slot(head h) = (h%2)*2 + h//2.
        vh = [
            pp.tile([128, H_PER_CORE, 2 * HD], BF16, tag=f"vh{b}", name=f"vh{b}")
            for b in range(N_KB)
        ]

        def xt_ap(kb, ct):
            return xt0[kb][:] if ct == 0 else xtc[ct - 1][:, kb, :]

        # ---- warm-up: pin pe_busy_start at ~0 so the 3us p-state ramp is
        # done before real data arrives (Pool memset: runs earliest)
        nc.gpsimd.memset(warm[:], 1.0)
        wps = mmps.tile([128, 512], F32, tag="mm512", name="warm_ps")
        nc.tensor.matmul(wps[:, 0:16], warm[:], warm[:, 0:16], start=True, stop=True)

        # ---- input DMAs (all SP-issued; HWDGE serializes at ~625ns/DMA).
        # Pair order feeds the kb-major tile-0 projection as it lands; wv
        # inserted early (first v matmuls need it ~5us in); the rest after.
        for kb in range(N_DIMB):
            nc.sync.dma_start(wqk[kb][:], wqkT[:, kb, :])
            nc.sync.dma_start(xt0[kb][:], xT[:, kb, 0:QT])
            if kb == 1:
                nc.sync.dma_start(wv[:], wvT[:])
        # tile-1 columns per-kb (stream into attend(0)'s proj(1) fillers)
        for kb in range(N_DIMB):
            nc.sync.dma_start(xtc[0][:, kb, :], xT[:, kb, QT : 2 * QT])
        for ct in range(2, N_QT):
            cs = slice(ct * QT, (ct + 1) * QT)
            nc.sync.dma_start(xtc[ct - 1][:], xT[:, :, cs])
        nc.sync.dma_start(wo[:], woT[:])

        def vh_fill(tb, src_ps):
            """src_ps: [128, 256] psum = v features for the 4 heads of this
            key block, column order [v_h0 v_h2 v_h1 v_h3]."""
            s4 = src_ps.rearrange("p (s c) -> p s c", s=4)
            nc.vector.memset(vh[tb][:, 0:2, HD : 2 * HD], 1.0)
            nc.vector.memset(vh[tb][:, 2:4, 0:HD], 1.0)
            nc.vector.tensor_copy(vh[tb][:, 0:2, 0:HD], s4[:, 0:2, :])
            nc.vector.tensor_copy(vh[tb][:, 2:4, HD : 2 * HD], s4[:, 2:4, :])

        def proj0():
            """Tile-0 q/k/v projections, kb-major: 8 accumulation chains in
            parallel across 4 psum tiles so each (wqk[kb], xt0[kb]) DMA pair
            is consumed as it arrives."""
            # one accumulation chain per PSUM bank: qk chains in 512-col bank
            # halves of the 2-bank s2 tiles, each v chain in its own 1-bank
            # tile (two concurrent chains in one bank are illegal)
            qk01 = sps.tile([128, 2 * QT], F32, tag="s2", name="p0qk01")
            qk23 = sps.tile([128, 2 * QT], F32, tag="s2", name="p0qk23")
            vps = [
                (mmps if tb < 2 else otps).tile(
                    [128, 256], F32, tag=("mm512" if tb < 2 else "ot"), name=f"p0v{tb}"
                )
                for tb in range(4)
            ]

            def v_mms(kb):
                for tb in range(4):
                    nc.tensor.matmul(
                        vps[tb][:],
                        xt0[kb][:, tb * KB : (tb + 1) * KB],
                        wv[:, kb, :],
                        start=(kb == 0),
                        stop=(kb == N_DIMB - 1),
                    )

            # v matmuls lag the qk stream by 3 kb so the wv DMA (issued 5th)
            # has landed before the first one fires
            for kb in range(N_DIMB):
                for ob in range(4):
                    ps = qk01 if ob < 2 else qk23
                    nc.tensor.matmul(
                        ps[:, (ob % 2) * QT : (ob % 2 + 1) * QT],
                        wqk[kb][:, ob * 128 : (ob + 1) * 128],
                        xt0[kb][:],
                        start=(kb == 0),
                        stop=(kb == N_DIMB - 1),
                    )
                if kb >= 3:
                    v_mms(kb - 3)
            for kb in range(N_DIMB - 3, N_DIMB):
                v_mms(kb)
            nc.vector.tensor_copy(qt[0][0][:], qk01[:, 0:QT])
            nc.vector.tensor_copy(qt[1][0][:], qk01[:, QT : 2 * QT])
            nc.vector.tensor_copy(kt[0][0][:], qk23[:, 0:QT])
            nc.vector.tensor_copy(kt[1][0][:], qk23[:, QT : 2 * QT])
            for tb in range(4):
                vh_fill(tb, vps[tb][:])

        def qk_chain(tt, ob):  # ob 0,1 -> q pair blocks; 2,3 -> k pair blocks
            ps = mmps.tile([128, 512], F32, tag="mm512", name=f"qk_ps{tt}_{ob}")
            for kb in range(N_DIMB):
                nc.tensor.matmul(
                    ps[:],
                    wqk[kb][:, ob * 128 : (ob + 1) * 128],
                    xt_ap(kb, tt),
                    start=(kb == 0),
                    stop=(kb == N_DIMB - 1),
                )
            dest = (qt if ob < 2 else kt)[ob % 2][tt]
            nc.vector.tensor_copy(dest[:], ps[:])

        def v_chain(tb):
            ps = mmps.tile([128, 256], F32, tag="mm512", name=f"v_ps{tb}")
            for kb in range(N_DIMB):
                nc.tensor.matmul(
                    ps[:],
                    xt_ap(kb, tb // 4)[:, (tb % 4) * KB : (tb % 4 + 1) * KB],
                    wv[:, kb, :],
                    start=(kb == 0),
                    stop=(kb == N_DIMB - 1),
                )
            vh_fill(tb, ps[:])

        def proj_pieces(tt):
            for ob in range(4):
                yield lambda ob=ob: qk_chain(tt, ob)
            for tb in range(4 * tt, 4 * tt + 4):
                yield lambda tb=tb: v_chain(tb)

        def attend(tt, os_pair, fillers, tail_fn=None, late=None):
            nb = 4 * (tt + 1)  # allowed key blocks for this query tile
            # pace fillers to finish ~3 steps early so their DVE copies have
            # drained before the final normalize needs the DVE
            n_steps = max(2 * nb - 3, 1)
            step = 0
            done_fill = 0
            n_fill = len(fillers)

            def fill():
                nonlocal done_fill
                want = (step + 1) * n_fill // n_steps
                while done_fill < want and fillers:
                    fillers.popleft()()
                    done_fill += 1

            for hp in range(2):  # head pair (2hp, 2hp+1)
                ot = [
                    otps.tile([128, QT], F32, tag="ot", name=f"ot{tt}_{hp}_{i}")
                    for i in range(2)
                ]

                def s_mm(b):
                    """S^T for key block b, both heads, into one 2-bank tile."""
                    diag = b - 4 * tt
                    d = diag * 128 if diag >= 0 else 0
                    s = sps.tile([128, 2 * QT], F32, tag="s2", name=f"s{tt}_{hp}_{b}")
                    for i in range(2):
                        rows = slice(i * 64, i * 64 + 64)
                        nc.tensor.matmul(
                            s[:, i * QT + d : (i + 1) * QT],
                            kt[hp][b // 4][rows, (b % 4) * KB : (b % 4 + 1) * KB],
                            qt[hp][tt][rows, d:QT],
                            start=True,
                            stop=True,
                        )
                    p = ppool.tile([128, 2 * QT], BF16, tag="p", name=f"p{tt}_{hp}_{b}")
                    return s, p

                s_tiles = {0: s_mm(0)}
                for b in range(nb):
                    if b + 1 < nb:
                        s_tiles[b + 1] = s_mm(b + 1)
                    diag = b - 4 * tt
                    d = diag * 128 if diag >= 0 else 0
                    s, p = s_tiles.pop(b)
                    if diag < 0:
                        nc.scalar.activation(p[:], s[:], EXP, scale=SCALE)
                    else:
                        # one exp for both heads over cols >= d (all rows),
                        # then zero the masked corner (rows 64-127 of each
                        # head attend only cols >= d+64) AFTER the exp
                        s2 = s[:].rearrange("p (h c) -> p h c", h=2)
                        p2 = p[:].rearrange("p (h c) -> p h c", h=2)
                        nc.scalar.activation(
                            p2[:, :, d:QT], s2[:, :, d:QT], EXP, scale=SCALE
                        )
                        nc.vector.memset(p2[64:128, :, d : d + 64], 0.0)
                    for i in range(2):
                        # slot order [h0, h2, h1, h3]: head 2hp+i -> slot 2i+hp
                        nc.tensor.matmul(
                            ot[i][:, d:QT],
                            vh[b][:, 2 * i + hp, :],
                            p[:, i * QT + d : (i + 1) * QT],
                            start=(b == 0),
                            stop=(b == nb - 1),
                        )
                    fill()
                    step += 1

                # normalize:
                #   ot[0] (even head) = [num | den]; ot[1] (odd) = [den | num]
                #   R[0:64] = 1/den0, R[64:128] = 1/den1 (shifted unary ok)
                #   os[0:64] = ot[0][0:64]*R[0:64]  (aligned)
                #   os[64:128] = ot[1][64:128]*R[64:128]  (aligned)
                if hp == 1 and late:
                    # PE work for the normalize window; their copies go to
                    # the Act engine (idle once the last exp is done) so the
                    # DVE normalize chain is not delayed
                    for piece in late:
                        piece()
                rb = rpool.tile([128, QT], F32, tag="rb", name=f"rb{tt}_{hp}")
                if tail_fn is not None and hp == 1:
                    # per-128-token normalize chunks, each immediately
                    # followed by that token slice's output projection
                    for t4 in range(4):
                        cs = slice(t4 * 128, (t4 + 1) * 128)
                        nc.vector.reciprocal(rb[0:64, cs], ot[0][64:128, cs])
                        nc.vector.reciprocal(rb[64:128, cs], ot[1][0:64, cs])
                        nc.vector.tensor_mul(
                            os_pair[hp][0:64, cs], ot[0][0:64, cs], rb[0:64, cs]
                        )
                        nc.vector.tensor_mul(
                            os_pair[hp][64:128, cs], ot[1][64:128, cs], rb[64:128, cs]
                        )
                        tail_fn(t4)
                else:
                    nc.vector.reciprocal(rb[0:64, :], ot[0][64:128, :])
                    nc.vector.reciprocal(rb[64:128, :], ot[1][0:64, :])
                    nc.vector.tensor_mul(
                        os_pair[hp][0:64, :], ot[0][0:64, :], rb[0:64, :]
                    )
                    nc.vector.tensor_mul(
                        os_pair[hp][64:128, :], ot[1][64:128, :], rb[64:128, :]
                    )

            while fillers:
                fillers.popleft()()

        def y_pieces(tt, os_pair, act_copy_from=99):
            """Output projection for query tile tt, run as attend fillers.
            Copies on DVE (the Act engine is exp-bound inside attends) except
            pieces >= act_copy_from, meant to run after the last exp."""
            pieces = []
            for t4 in range(4):
                trows = slice(t4 * 128, (t4 + 1) * 128)
                ysb = ypool.tile([128, DIM], BF16, tag="ysb", name=f"ysb{tt}_{t4}")
                for jb in range(2):
                    idx = 2 * t4 + jb

                    def piece(t4=t4, jb=jb, ysb=ysb, trows=trows, idx=idx):
                        yps = mmps.tile(
                            [128, 512], F32, tag="mm512", name=f"y_ps{tt}_{t4}_{jb}"
                        )
                        for db in range(2):
                            nc.tensor.matmul(
                                yps[:],
                                os_pair[db][:, trows],
                                wo[:, db, jb * 512 : (jb + 1) * 512],
                                start=(db == 0),
                                stop=(db == 1),
                            )
                        dest = ysb[:, jb * 512 : (jb + 1) * 512]
                        if idx >= act_copy_from:
                            nc.scalar.activation(dest, yps[:], COPY)
                        else:
                            nc.vector.tensor_copy(dest, yps[:])
                        if jb == 1:
                            r0 = tt * QT + t4 * 128
                            nc.sync.dma_start(y[r0 : r0 + 128, :], ysb[:])

                    pieces.append(piece)
            return pieces

        def y_tail_piece(tt, os_pair, t4):
            """Final tile's output projection for one 128-token slice: both
            jb halves into one 2-bank s_ps tile (free after the last attend),
            halves copied out on alternating DVE/Act engines, DMA per half."""
            trows = slice(t4 * 128, (t4 + 1) * 128)
            ysb = ypool.tile([128, DIM], BF16, tag="ytb", name=f"yt{tt}_{t4}")
            yps = sps.tile([128, 2 * QT], F32, tag="s2", name=f"yt_ps{tt}_{t4}")
            for jb in range(2):
                for db in range(2):
                    nc.tensor.matmul(
                        yps[:, jb * 512 : (jb + 1) * 512],
                        os_pair[db][:, trows],
                        wo[:, db, jb * 512 : (jb + 1) * 512],
                        start=(db == 0),
                        stop=(db == 1),
                    )
            # single full-width Act copy + DMA: keeps the DVE free for the
            # normalize chain, which paces this tail
            r0 = tt * QT + t4 * 128
            nc.scalar.activation(ysb[:], yps[:], COPY)
            nc.sync.dma_start(y[r0 : r0 + 128, :], ysb[:])

        # ---- the pipeline ----
        # attend(0) <- proj(1); attend(1) <- proj(2)+y(0); attend(2) <-
        # proj(3); attend(3) <- y(1)+y(2) (reserved: the last attend has no
        # next-tile projections to hide its exp latency behind); y(3) tail.
        def interleave(a, b):
            out = deque()
            a, b = deque(a), deque(b)
            while a or b:
                if b:
                    out.append(b.popleft())
                if a:
                    out.append(a.popleft())
            return out

        proj0()
        os_all = []
        for tt in range(N_QT):
            os_all.append(
                [
                    ospool.tile([128, QT], BF16, tag=f"os{i}_{tt}", name=f"os{i}_{tt}")
                    for i in range(2)
                ]
            )

        attend(0, os_all[0], deque(proj_pieces(1)))
        y0 = y_pieces(0, os_all[0])
        attend(1, os_all[1], interleave(proj_pieces(2), y0))
        y1 = y_pieces(1, os_all[1])
        attend(2, os_all[2], deque(proj_pieces(3)))
        y2 = y_pieces(2, os_all[2], act_copy_from=6)
        attend(
            3,
            os_all[3],
            deque(y1 + y2[:6]),
            tail_fn=lambda t4: y_tail_piece(3, os_all[3], t4),
            late=y2[6:],
        )


def build():
    global _CACHED_NC
    if _CACHED_NC is not None:
        return _CACHED_NC
    nc = bacc.Bacc(
        "TRN2", target_bir_lowering=False, debug=False, enable_asserts=False
    )
    xT = nc.dram_tensor("xT", [128, N_DIMB, T], BF16, kind="ExternalInput").ap()
    wqkT = nc.dram_tensor("wqkT", [128, N_DIMB, 512], BF16, kind="ExternalInput").ap()
    wvT = nc.dram_tensor("wvT", [128, N_DIMB, 256], BF16, kind="ExternalInput").ap()
    woT = nc.dram_tensor("woutT", [128, 2, DIM], BF16, kind="ExternalInput").ap()
    y = nc.dram_tensor("y", [T, DIM], BF16, kind="ExternalOutput").ap()
    with tile.TileContext(nc) as tc:
        _emit(nc, tc, xT, wqkT, wvT, woT, y)
    nc.compile()
    _CACHED_NC = nc
    return nc


def _to_bf16_3d(mat2d, inner):
    """[R, C] f32 -> [128, R//128, C] bf16 with row index (kb*128+p) -> [p, kb]."""
    import ml_dtypes

    r, c = mat2d.shape
    assert r % 128 == 0 and c == inner
    return np.ascontiguousarray(
        mat2d.reshape(r // 128, 128, c).transpose(1, 0, 2)
    ).astype(ml_dtypes.bfloat16)


def make_in_maps(x, Wqkv, Wout):
    """Host-side sharding: core c = (batch c//4, head-group c%4)."""
    in_maps = []
    for c in range(8):
        b, hg = divmod(c, 4)
        hs = hg * H_PER_CORE
        r0, r1 = hs * HD, (hs + H_PER_CORE) * HD
        qrows = Wqkv[r0:r1]
        krows = Wqkv[DIM + r0 : DIM + r1]
        vrows = Wqkv[2 * DIM + r0 : 2 * DIM + r1]
        # v head blocks reordered [h0, h2, h1, h3] to match the vh slot order
        vperm = np.concatenate(
            [vrows[0:64], vrows[128:192], vrows[64:128], vrows[192:256]], 0
        )
        in_maps.append(
            {
                "xT": _to_bf16_3d(np.ascontiguousarray(x[b].T), T),
                "wqkT": _to_bf16_3d(
                    np.ascontiguousarray(np.concatenate([qrows, krows], 0).T), 512
                ),
                "wvT": _to_bf16_3d(np.ascontiguousarray(vperm.T), 256),
                "woutT": _to_bf16_3d(np.ascontiguousarray(Wout[:, r0:r1].T), DIM),
            }
        )
    return in_maps


def kernel(x, Wqkv, Wout):
    x = np.asarray(x, dtype=np.float32)
    Wqkv = np.asarray(Wqkv, dtype=np.float32)
    Wout = np.asarray(Wout, dtype=np.float32)
    nc = build()
    in_maps = make_in_maps(x, Wqkv, Wout)
    res = run_bass_kernel_spmd(nc, in_maps, core_ids=list(range(8)))
    out = np.zeros((B, T, DIM), np.float32)
    for c in range(8):
        out[c // 4] += res.results[c]["y"].astype(np.float32)
    return out


# revision 45
# speedup vs baseline: 1.1445x; 1.0147x over previous
"""Trainium2 Bass kernel for block-causal (chunked) multi-head attention.

Computes, for x:[2,2048,1024], Wqkv:[3072,1024], Wout:[1024,1024]:
    qkv = x @ Wqkv.T ; per-head scaled scores; block-causal mask
    (causal OR same 64-chunk == full attention to all chunks <= own chunk);
    softmax; out = attn @ v ; y = out @ Wout.T

Sharding over 8 NeuronCores: data-parallel over batch (2) x tensor-parallel
over heads (16 heads -> 4 per core).  Each core projects q/k/v for its 4
heads, runs attention, and computes a partial output projection against its
256 columns of Wout; the host sums the 4 partials per batch element.

v2 vs v1:
  * All matmul operands in bf16 (PE cost model: 1.0 cycles/row at any free
    size, vs fp32r's 4x penalty below 256) and all input DMA halved.
  * A tiny warm-up matmul at t~0 pins pe_busy_start so every real matmul
    after ~3us runs at the full 2.4 GHz p-state.
  * Tile-0 projections are emitted kb-major (all 8 psum chains accumulate in
    parallel) so the PE consumes each (wqk[kb], xt[kb]) DMA pair as it lands
    instead of stalling on a chain-major walk.
  * vh slot order [h0, h2 | h1, h3] with the ones-columns swapped to the
    low half for odd heads, so the softmax normalization needs no
    partition-shifted DVE copy: both heads' numerators land mul-aligned
    with their reciprocals.
  * y tiles are copied out of PSUM on alternating DVE/Act engines and
    DMA'd per 512-half; the last tile's normalize is split per 128-token
    slice so its y matmuls start before the full mul finishes.

Scores are computed transposed (S^T[tk, tq]) so that the attention matmul
needs no transposes, and the softmax denominator comes from ones-columns
interleaved in V.  The block-causal mask is realized structurally: masked
key blocks are never computed, and diagonal blocks use rectangular
sub-views (chunk granularity 64) with one small memset for the corner.
"""

import sys

if "/opt/trn_rl_repo" not in sys.path:
    sys.path.insert(0, "/opt/trn_rl_repo")

from collections import deque

import numpy as np

import concourse.bass as bass  # noqa: F401  (registers types)
import concourse.mybir as mybir
import concourse.tile as tile
from concourse import bacc
from concourse.bass_utils import run_bass_kernel_spmd

F32 = mybir.dt.float32
BF16 = mybir.dt.bfloat16
EXP = mybir.ActivationFunctionType.Exp
COPY = mybir.ActivationFunctionType.Copy

B = 2
T = 2048
DIM = 1024
N_HEADS = 16
HD = 64
CHUNK = 64
H_PER_CORE = 4  # 16 heads / (8 cores / 2 batches)
QT = 512  # query tile (free dim of S^T matmuls)
KB = 128  # key block (contraction block of AV matmuls)
N_QT = T // QT  # 4
N_KB = T // KB  # 16
N_DIMB = DIM // 128  # 8 contraction blocks for the projections
SCALE = 1.0 / np.sqrt(HD)

_CACHED_NC = None


def _emit(nc, tc, xT, wqkT, wvT, woT, y):
    po = tc.tile_pool  # shorthand

    with (
        po(name="persist", bufs=1) as pp,
        po(name="s_ps", bufs=2, space="PSUM") as sps,  # [128,1024] score slots
        po(name="mm_ps", bufs=2, space="PSUM") as mmps,  # [128,512] proj/y slots
        po(name="ot_ps", bufs=2, space="PSUM") as otps,  # [128,512] outT slots
        po(name="pbuf", bufs=6) as ppool,  # exp(S^T) tiles
        po(name="osbuf", bufs=2) as ospool,  # assembled normalized outT
        po(name="rbuf", bufs=2) as rpool,  # reciprocal denominators
        po(name="ybuf", bufs=6) as ypool,
    ):
        # ---- persistent SBUF tensors ----
        warm = pp.tile([128, 128], BF16, tag="warm", name="warm")
        # xt for column-tile 0: per-kb tiles (fine DMA deps feed the kb-major
        # projection); tiles 1-3: one [128, 8, 512] tile each, single DMA
        xt0 = [pp.tile([128, QT], BF16, tag=f"xt0_{k}", name=f"xt0_{k}") for k in range(N_DIMB)]
        xtc = [
            pp.tile([128, N_DIMB, QT], BF16, tag=f"xtc{c}", name=f"xtc{c}")
            for c in range(1, N_QT)
        ]
        wqk = [pp.tile([128, 512], BF16, tag=f"wqk{k}", name=f"wqk{k}") for k in range(N_DIMB)]
        wv = pp.tile([128, N_DIMB, 256], BF16, tag="wv", name="wv")
        wo = pp.tile([128, 2, DIM], BF16, tag="wo", name="wo")
        # q/k head-dim-major: partition block hp holds heads (2hp, 2hp+1)
        qt = [
            [pp.tile([128, QT], BF16, tag=f"qt{i}_{c}", name=f"qt{i}_{c}") for c in range(N_QT)]
            for i in range(2)
        ]
        kt = [
            [pp.tile([128, QT], BF16, tag=f"kt{i}_{c}", name=f"kt{i}_{c}") for c in range(N_QT)]
            for i in range(2)
        ]
        # v (token-major) + ones columns, per key block: [128, slot, 128]
        # slot order is [h0, h2, h1, h3]; even-slot layout [v | ones],
        # odd-head layout [ones | v] (so AV's denominator lands where the
        # normalize mul wants it).  slot(head h) = (h%2)*2 + h//2.
        vh = [
            pp.tile([128, H_PER_CORE, 2 * HD], BF16, tag=f"vh{b}", name=f"vh{b}")
            for b in range(N_KB)
        ]

        def xt_ap(kb, ct):
            return xt0[kb][:] if ct == 0 else xtc[ct - 1][:, kb, :]

        # ---- warm-up: pin pe_busy_start at ~0 so the 3us p-state ramp is
        # done before real data arrives (Pool memset: runs earliest)
        nc.gpsimd.memset(warm[:], 1.0)
        wps = mmps.tile([128, 512], F32, tag="mm512", name="warm_ps")
        nc.tensor.matmul(wps[:, 0:16], warm[:], warm[:, 0:16], start=True, stop=True)

        # ---- input DMAs (all SP-issued; HWDGE serializes at ~625ns/DMA).
        # Pair order feeds the kb-major tile-0 projection as it lands; wv
        # inserted early (first v matmuls need it ~5us in); the rest after.
        # xt0[0] and wv go through the Pool engine's SWDGE path — a second,
        # parallel DMA issue pipe (HWDGE serializes at ~625ns/DMA)
        nc.gpsimd.dma_start(xt0[0][:], xT[:, 0, 0:QT])
        nc.gpsimd.dma_start(wv[:], wvT[:])
        for kb in range(N_DIMB):
            nc.sync.dma_start(wqk[kb][:], wqkT[:, kb, :])
            if kb > 0:
                nc.sync.dma_start(xt0[kb][:], xT[:, kb, 0:QT])
        # tile-1 columns per-kb (stream into attend(0)'s proj(1) fillers)
        for kb in range(N_DIMB):
            nc.sync.dma_start(xtc[0][:, kb, :], xT[:, kb, QT : 2 * QT])
        for ct in range(2, N_QT):
            cs = slice(ct * QT, (ct + 1) * QT)
            nc.sync.dma_start(xtc[ct - 1][:], xT[:, :, cs])
        nc.sync.dma_start(wo[:], woT[:])

        def vh_fill(tb, src_ps):
            """src_ps: [128, 256] psum = v features for the 4 heads of this
            key block, column order [v_h0 v_h2 v_h1 v_h3]."""
            s4 = src_ps.rearrange("p (s c) -> p s c", s=4)
            nc.vector.memset(vh[tb][:, 0:2, HD : 2 * HD], 1.0)
            nc.vector.memset(vh[tb][:, 2:4, 0:HD], 1.0)
            nc.vector.tensor_copy(vh[tb][:, 0:2, 0:HD], s4[:, 0:2, :])
            nc.vector.tensor_copy(vh[tb][:, 2:4, HD : 2 * HD], s4[:, 2:4, :])

        def proj0():
            """Tile-0 q/k/v projections, kb-major: 8 accumulation chains in
            parallel across 4 psum tiles so each (wqk[kb], xt0[kb]) DMA pair
            is consumed as it arrives."""
            # one accumulation chain per PSUM bank: qk chains in 512-col bank
            # halves of the 2-bank s2 tiles, each v chain in its own 1-bank
            # tile (two concurrent chains in one bank are illegal)
            qk01 = sps.tile([128, 2 * QT], F32, tag="s2", name="p0qk01")
            qk23 = sps.tile([128, 2 * QT], F32, tag="s2", name="p0qk23")
            vps = [
                (mmps if tb < 2 else otps).tile(
                    [128, 256], F32, tag=("mm512" if tb < 2 else "ot"), name=f"p0v{tb}"
                )
                for tb in range(4)
            ]

            def v_mms(kb):
                for tb in range(4):
                    nc.tensor.matmul(
                        vps[tb][:],
                        xt0[kb][:, tb * KB : (tb + 1) * KB],
                        wv[:, kb, :],
                        start=(kb == 0),
                        stop=(kb == N_DIMB - 1),
                    )

            # v matmuls lag the qk stream by 3 kb so the wv DMA (issued 5th)
            # has landed before the first one fires
            for kb in range(N_DIMB):
                for ob in range(4):
                    ps = qk01 if ob < 2 else qk23
                    nc.tensor.matmul(
                        ps[:, (ob % 2) * QT : (ob % 2 + 1) * QT],
                        wqk[kb][:, ob * 128 : (ob + 1) * 128],
                        xt0[kb][:],
                        start=(kb == 0),
                        stop=(kb == N_DIMB - 1),
                    )
                if kb >= 3:
                    v_mms(kb - 3)
            for kb in range(N_DIMB - 3, N_DIMB):
                v_mms(kb)
            nc.vector.tensor_copy(qt[0][0][:], qk01[:, 0:QT])
            nc.vector.tensor_copy(qt[1][0][:], qk01[:, QT : 2 * QT])
            nc.vector.tensor_copy(kt[0][0][:], qk23[:, 0:QT])
            nc.vector.tensor_copy(kt[1][0][:], qk23[:, QT : 2 * QT])
            for tb in range(4):
                vh_fill(tb, vps[tb][:])

        def qk_chain(tt, ob):  # ob 0,1 -> q pair blocks; 2,3 -> k pair blocks
            ps = mmps.tile([128, 512], F32, tag="mm512", name=f"qk_ps{tt}_{ob}")
            for kb in range(N_DIMB):
                nc.tensor.matmul(
                    ps[:],
                    wqk[kb][:, ob * 128 : (ob + 1) * 128],
                    xt_ap(kb, tt),
                    start=(kb == 0),
                    stop=(kb == N_DIMB - 1),
                )
            dest = (qt if ob < 2 else kt)[ob % 2][tt]
            nc.vector.tensor_copy(dest[:], ps[:])

        def v_chain(tb):
            ps = mmps.tile([128, 256], F32, tag="mm512", name=f"v_ps{tb}")
            for kb in range(N_DIMB):
                nc.tensor.matmul(
                    ps[:],
                    xt_ap(kb, tb // 4)[:, (tb % 4) * KB : (tb % 4 + 1) * KB],
                    wv[:, kb, :],
                    start=(kb == 0),
                    stop=(kb == N_DIMB - 1),
                )
            vh_fill(tb, ps[:])

        def proj_pieces(tt):
            for ob in range(4):
                yield lambda ob=ob: qk_chain(tt, ob)
            for tb in range(4 * tt, 4 * tt + 4):
                yield lambda tb=tb: v_chain(tb)

        def attend(tt, os_pair, fillers, tail_fn=None, late=None):
            nb = 4 * (tt + 1)  # allowed key blocks for this query tile
            # pace fillers to finish ~3 steps early so their DVE copies have
            # drained before the final normalize needs the DVE
            n_steps = 2 * nb
            step = 0
            done_fill = 0
            n_fill = len(fillers)

            def fill():
                nonlocal done_fill
                want = (step + 1) * n_fill // n_steps
                while done_fill < want and fillers:
                    fillers.popleft()()
                    done_fill += 1

            for hp in range(2):  # head pair (2hp, 2hp+1)
                ot = [
                    otps.tile([128, QT], F32, tag="ot", name=f"ot{tt}_{hp}_{i}")
                    for i in range(2)
                ]

                def s_mm(b):
                    """S^T for key block b, both heads, into one 2-bank tile."""
                    diag = b - 4 * tt
                    d = diag * 128 if diag >= 0 else 0
                    s = sps.tile([128, 2 * QT], F32, tag="s2", name=f"s{tt}_{hp}_{b}")
                    for i in range(2):
                        rows = slice(i * 64, i * 64 + 64)
                        nc.tensor.matmul(
                            s[:, i * QT + d : (i + 1) * QT],
                            kt[hp][b // 4][rows, (b % 4) * KB : (b % 4 + 1) * KB],
                            qt[hp][tt][rows, d:QT],
                            start=True,
                            stop=True,
                        )
                    p = ppool.tile([128, 2 * QT], BF16, tag="p", name=f"p{tt}_{hp}_{b}")
                    return s, p

                s_tiles = {0: s_mm(0)}
                for b in range(nb):
                    if b + 1 < nb:
                        s_tiles[b + 1] = s_mm(b + 1)
                    diag = b - 4 * tt
                    d = diag * 128 if diag >= 0 else 0
                    s, p = s_tiles.pop(b)
                    if diag < 0:
                        nc.scalar.activation(p[:], s[:], EXP, scale=SCALE)
                    else:
                        # one exp for both heads over cols >= d (all rows),
                        # then zero the masked corner (rows 64-127 of each
                        # head attend only cols >= d+64) AFTER the exp
                        s2 = s[:].rearrange("p (h c) -> p h c", h=2)
                        p2 = p[:].rearrange("p (h c) -> p h c", h=2)
                        nc.scalar.activation(
                            p2[:, :, d:QT], s2[:, :, d:QT], EXP, scale=SCALE
                        )
                        nc.vector.memset(p2[64:128, :, d : d + 64], 0.0)
                    for i in range(2):
                        # slot order [h0, h2, h1, h3]: head 2hp+i -> slot 2i+hp
                        nc.tensor.matmul(
                            ot[i][:, d:QT],
                            vh[b][:, 2 * i + hp, :],
                            p[:, i * QT + d : (i + 1) * QT],
                            start=(b == 0),
                            stop=(b == nb - 1),
                        )
                    fill()
                    step += 1

                # normalize:
                #   ot[0] (even head) = [num | den]; ot[1] (odd) = [den | num]
                #   R[0:64] = 1/den0, R[64:128] = 1/den1 (shifted unary ok)
                #   os[0:64] = ot[0][0:64]*R[0:64]  (aligned)
                #   os[64:128] = ot[1][64:128]*R[64:128]  (aligned)
                if hp == 1 and late:
                    # PE work for the normalize window; their copies go to
                    # the Act engine (idle once the last exp is done) so the
                    # DVE normalize chain is not delayed
                    for piece in late:
                        piece()
                rb = rpool.tile([128, QT], F32, tag="rb", name=f"rb{tt}_{hp}")
                if tail_fn is not None and hp == 1:
                    # per-128-token normalize chunks, each immediately
                    # followed by that token slice's output projection; the
                    # os0-only (db=0) halves of the first two slices are
                    # opened pre-norm so the PE has work during the first
                    # normalize ops
                    tail_open, tail_close = tail_fn
                    yopen = {t4: tail_open(t4) for t4 in range(2)}
                    for t4 in range(4):
                        cs = slice(t4 * 128, (t4 + 1) * 128)
                        nc.vector.reciprocal(rb[0:64, cs], ot[0][64:128, cs])
                        nc.vector.reciprocal(rb[64:128, cs], ot[1][0:64, cs])
                        nc.vector.tensor_mul(
                            os_pair[hp][0:64, cs], ot[0][0:64, cs], rb[0:64, cs]
                        )
                        nc.vector.tensor_mul(
                            os_pair[hp][64:128, cs], ot[1][64:128, cs], rb[64:128, cs]
                        )
                        tail_close(t4, yopen.pop(t4))
                        if t4 + 2 < 4:
                            yopen[t4 + 2] = tail_open(t4 + 2)
                else:
                    # per-head op pairs: ot[0]'s reads finish after two ops,
                    # releasing its psum slot for the next head-pair's AV
                    nc.vector.reciprocal(rb[0:64, :], ot[0][64:128, :])
                    nc.vector.tensor_mul(
                        os_pair[hp][0:64, :], ot[0][0:64, :], rb[0:64, :]
                    )
                    nc.vector.reciprocal(rb[64:128, :], ot[1][0:64, :])
                    nc.vector.tensor_mul(
                        os_pair[hp][64:128, :], ot[1][64:128, :], rb[64:128, :]
                    )

            while fillers:
                fillers.popleft()()

        def y_pieces(tt, os_pair, act_copy_from=99):
            """Output projection for query tile tt, run as attend fillers.
            Copies on DVE (the Act engine is exp-bound inside attends) except
            pieces >= act_copy_from, meant to run after the last exp."""
            pieces = []
            for t4 in range(4):
                trows = slice(t4 * 128, (t4 + 1) * 128)
                ysb = ypool.tile([128, DIM], BF16, tag="ysb", name=f"ysb{tt}_{t4}")
                for jb in range(2):
                    idx = 2 * t4 + jb

                    def piece(t4=t4, jb=jb, ysb=ysb, trows=trows, idx=idx):
                        yps = mmps.tile(
                            [128, 512], F32, tag="mm512", name=f"y_ps{tt}_{t4}_{jb}"
                        )
                        for db in range(2):
                            nc.tensor.matmul(
                                yps[:],
                                os_pair[db][:, trows],
                                wo[:, db, jb * 512 : (jb + 1) * 512],
                                start=(db == 0),
                                stop=(db == 1),
                            )
                        dest = ysb[:, jb * 512 : (jb + 1) * 512]
                        if idx >= act_copy_from:
                            nc.scalar.activation(dest, yps[:], COPY)
                        else:
                            nc.vector.tensor_copy(dest, yps[:])
                        if jb == 1:
                            r0 = tt * QT + t4 * 128
                            nc.sync.dma_start(y[r0 : r0 + 128, :], ysb[:])

                    pieces.append(piece)
            return pieces

        def y_tail_open(tt, os_pair, t4):
            """Start the final tile's output projection for one 128-token
            slice: the db=0 (first head-pair, normalized mid-attend) matmuls
            of both jb halves into one 2-bank s_ps tile.  These only need
            os_pair[0], so they can run while the DVE normalizes os_pair[1]."""
            trows = slice(t4 * 128, (t4 + 1) * 128)
            yps = sps.tile([128, 2 * QT], F32, tag="s2", name=f"yt_ps{tt}_{t4}")
            for jb in range(2):
                nc.tensor.matmul(
                    yps[:, jb * 512 : (jb + 1) * 512],
                    os_pair[0][:, trows],
                    wo[:, 0, jb * 512 : (jb + 1) * 512],
                    start=True,
                    stop=False,
                )
            return yps

        def y_tail_close(tt, os_pair, t4, yps):
            """Finish a tail slice: db=1 accumulation, copy out, DMA."""
            trows = slice(t4 * 128, (t4 + 1) * 128)
            ysb = ypool.tile([128, DIM], BF16, tag="ytb", name=f"yt{tt}_{t4}")
            for jb in range(2):
                nc.tensor.matmul(
                    yps[:, jb * 512 : (jb + 1) * 512],
                    os_pair[1][:, trows],
                    wo[:, 1, jb * 512 : (jb + 1) * 512],
                    start=False,
                    stop=True,
                )
            # full-width Act copy + DMA keeps the DVE free for the normalize
            # chain, which paces this tail; the very last slice splits its
            # copy across DVE+Act halves so the final DMA starts sooner
            r0 = tt * QT + t4 * 128
            if t4 < 3:
                nc.scalar.activation(ysb[:], yps[:], COPY)
                nc.sync.dma_start(y[r0 : r0 + 128, :], ysb[:])
            else:
                # separate half tiles: tile-granular WAW tracking would
                # otherwise serialize the two engines' copies
                y2a = ypool.tile([128, 512], BF16, tag="ytb2a", name=f"yt2a{tt}")
                y2b = ypool.tile([128, 512], BF16, tag="ytb2b", name=f"yt2b{tt}")
                nc.vector.tensor_copy(y2a[:], yps[:, 0:512])
                nc.sync.dma_start(y[r0 : r0 + 128, 0:512], y2a[:])
                nc.scalar.activation(y2b[:], yps[:, 512:1024], COPY)
                nc.sync.dma_start(y[r0 : r0 + 128, 512:1024], y2b[:])

        # ---- the pipeline ----
        # attend(0) <- proj(1); attend(1) <- proj(2)+y(0); attend(2) <-
        # proj(3); attend(3) <- y(1)+y(2) (reserved: the last attend has no
        # next-tile projections to hide its exp latency behind); y(3) tail.
        def interleave(a, b):
            out = deque()
            a, b = deque(a), deque(b)
            while a or b:
                if b:
                    out.append(b.popleft())
                if a:
                    out.append(a.popleft())
            return out

        proj0()
        os_all = []
        for tt in range(N_QT):
            os_all.append(
                [
                    ospool.tile([128, QT], BF16, tag=f"os{i}_{tt}", name=f"os{i}_{tt}")
                    for i in range(2)
                ]
            )

        attend(0, os_all[0], deque(proj_pieces(1)))
        y0 = y_pieces(0, os_all[0])
        attend(1, os_all[1], interleave(proj_pieces(2), y0))
        y1 = y_pieces(1, os_all[1])
        attend(2, os_all[2], deque(proj_pieces(3)))
        y2 = y_pieces(2, os_all[2], act_copy_from=6)
        attend(
            3,
            os_all[3],
            deque(y1 + y2[:6]),
            tail_fn=(
                lambda t4: y_tail_open(3, os_all[3], t4),
                lambda t4, yps: y_tail_close(3, os_all[3], t4, yps),
            ),
            late=y2[6:],
        )


def build():
    global _CACHED_NC
    if _CACHED_NC is not None:
        return _CACHED_NC
    nc = bacc.Bacc(
        "TRN2", target_bir_lowering=False, debug=False, enable_asserts=False
    )
    xT = nc.dram_tensor("xT", [128, N_DIMB, T], BF16, kind="ExternalInput").ap()
    wqkT = nc.dram_tensor("wqkT", [128, N_DIMB, 512], BF16, kind="ExternalInput").ap()
    wvT = nc.dram_tensor("wvT", [128, N_DIMB, 256], BF16, kind="ExternalInput").ap()
    woT = nc.dram_tensor("woutT", [128, 2, DIM], BF16, kind="ExternalInput").ap()
    y = nc.dram_tensor("y", [T, DIM], BF16, kind="ExternalOutput").ap()
    with tile.TileContext(nc) as tc:
        _emit(nc, tc, xT, wqkT, wvT, woT, y)
    nc.compile()
    _CACHED_NC = nc
    return nc


def _to_bf16_3d(mat2d, inner):
    """[R, C] f32 -> [128, R//128, C] bf16 with row index (kb*128+p) -> [p, kb]."""
    import ml_dtypes

    r, c = mat2d.shape
    assert r % 128 == 0 and c == inner
    return np.ascontiguousarray(
        mat2d.reshape(r // 128, 128, c).transpose(1, 0, 2)
    ).astype(ml_dtypes.bfloat16)


def make_in_maps(x, Wqkv, Wout):
    """Host-side sharding: core c = (batch c//4, head-group c%4)."""
    in_maps = []
    for c in range(8):
        b, hg = divmod(c, 4)
        hs = hg * H_PER_CORE
        r0, r1 = hs * HD, (hs + H_PER_CORE) * HD
        qrows = Wqkv[r0:r1]
        krows = Wqkv[DIM + r0 : DIM + r1]
        vrows = Wqkv[2 * DIM + r0 : 2 * DIM + r1]
        # v head blocks reordered [h0, h2, h1, h3] to match the vh slot order
        vperm = np.concatenate(
            [vrows[0:64], vrows[128:192], vrows[64:128], vrows[192:256]], 0
        )
        in_maps.append(
            {
                "xT": _to_bf16_3d(np.ascontiguousarray(x[b].T), T),
                "wqkT": _to_bf16_3d(
                    np.ascontiguousarray(np.concatenate([qrows, krows], 0).T), 512
                ),
                "wvT": _to_bf16_3d(np.ascontiguousarray(vperm.T), 256),
                "woutT": _to_bf16_3d(np.ascontiguousarray(Wout[:, r0:r1].T), DIM),
            }
        )
    return in_maps


def kernel(x, Wqkv, Wout):
    x = np.asarray(x, dtype=np.float32)
    Wqkv = np.asarray(Wqkv, dtype=np.float32)
    Wout = np.asarray(Wout, dtype=np.float32)
    nc = build()
    in_maps = make_in_maps(x, Wqkv, Wout)
    res = run_bass_kernel_spmd(nc, in_maps, core_ids=list(range(8)))
    out = np.zeros((B, T, DIM), np.float32)
    for c in range(8):
        out[c // 4] += res.results[c]["y"].astype(np.float32)
    return out


# revision 54
# speedup vs baseline: 1.1457x; 1.0010x over previous
"""Trainium2 Bass kernel for block-causal (chunked) multi-head attention.

Computes, for x:[2,2048,1024], Wqkv:[3072,1024], Wout:[1024,1024]:
    qkv = x @ Wqkv.T ; per-head scaled scores; block-causal mask
    (causal OR same 64-chunk == full attention to all chunks <= own chunk);
    softmax; out = attn @ v ; y = out @ Wout.T

Sharding over 8 NeuronCores: data-parallel over batch (2) x tensor-parallel
over heads (16 heads -> 4 per core).  Each core projects q/k/v for its 4
heads, runs attention, and computes a partial output projection against its
256 columns of Wout; the host sums the 4 partials per batch element.

v2 vs v1 (149us -> 130us in the TimelineSim cost model):
  * All matmul operands in bf16 (PE cost model: 1.0 cycles/row at any free
    size, vs fp32r's 4x penalty below 256 free) and all input/output DMA
    halved.  Host casts inputs to bf16; y partials return as bf16 and are
    summed on the host in f32 (rel_l2 ~6e-3 vs the 2e-2 gate).
  * A tiny warm-up matmul at t~0 pins pe_busy_start so every real matmul
    after ~3us runs at the full 2.4 GHz p-state.
  * Tile-0 projections are emitted kb-major: all 8 accumulation chains (4 qk
    + 4 v) run in parallel across psum banks (one chain per 2KB bank — two
    open groups in one bank are illegal), so the PE consumes each
    (wqk[kb], xt0[kb]) DMA pair as it lands instead of stalling on a
    chain-major walk.  xt0[0]/wv issue via the Pool SWDGE pipe, bypassing
    the serial ~625ns/DMA HWDGE; later x tiles are coarse multi-kb DMAs.
  * vh slot order [h0, h2 | h1, h3] with the ones-columns swapped to the
    low half for odd heads, so the softmax normalization needs no
    partition-shifted DVE copy: both heads' numerators land mul-aligned
    with their reciprocals.
  * Filler schedule: attend(0) <- proj(1); attend(1) <- proj(2)+y(0);
    attend(2) <- proj(3); attend(3) <- y(1)+y(2) (the last attend has no
    next-tile projections to hide its exp latency behind), with two y(2)
    pieces held back for the final normalize window.  y-filler copies stay
    on the DVE (Act is exp-bound inside attends).
  * The last tile's y projection: normalize is chunked per 128-token slice
    interleaved with the y matmuls; the os0-only halves open their psum
    accumulation groups before the normalize so the PE stays fed; copies go
    out full-width on Act (DVE paces the normalize), final slice split
    DVE/Act in parallel.

Scores are computed transposed (S^T[tk, tq]) so that the attention matmul
needs no transposes, and the softmax denominator comes from ones-columns
interleaved in V.  The block-causal mask is realized structurally: masked
key blocks are never computed, and diagonal blocks use rectangular
sub-views (chunk granularity 64) with one small memset for the corner.
"""

import sys

if "/opt/trn_rl_repo" not in sys.path:
    sys.path.insert(0, "/opt/trn_rl_repo")

from collections import deque

import numpy as np

import concourse.bass as bass  # noqa: F401  (registers types)
import concourse.mybir as mybir
import concourse.tile as tile
from concourse import bacc
from concourse.bass_utils import run_bass_kernel_spmd

F32 = mybir.dt.float32
BF16 = mybir.dt.bfloat16
EXP = mybir.ActivationFunctionType.Exp
COPY = mybir.ActivationFunctionType.Copy

B = 2
T = 2048
DIM = 1024
N_HEADS = 16
HD = 64
CHUNK = 64
H_PER_CORE = 4  # 16 heads / (8 cores / 2 batches)
QT = 512  # query tile (free dim of S^T matmuls)
KB = 128  # key block (contraction block of AV matmuls)
N_QT = T // QT  # 4
N_KB = T // KB  # 16
N_DIMB = DIM // 128  # 8 contraction blocks for the projections
SCALE = 1.0 / np.sqrt(HD)

_CACHED_NC = None


def _emit(nc, tc, xT, wqkT, wvT, woT, y):
    po = tc.tile_pool  # shorthand

    with (
        po(name="persist", bufs=1) as pp,
        po(name="s_ps", bufs=2, space="PSUM") as sps,  # [128,1024] score slots
        po(name="mm_ps", bufs=2, space="PSUM") as mmps,  # [128,512] proj/y slots
        po(name="ot_ps", bufs=2, space="PSUM") as otps,  # [128,512] outT slots
        po(name="pbuf", bufs=8) as ppool,  # exp(S^T) tiles
        po(name="osbuf", bufs=2) as ospool,  # assembled normalized outT
        po(name="rbuf", bufs=4) as rpool,  # reciprocal denominators
        po(name="ybuf", bufs=6) as ypool,
    ):
        # ---- persistent SBUF tensors ----
        warm = pp.tile([128, 128], BF16, tag="warm", name="warm")
        # xt for column-tile 0: per-kb tiles (fine DMA deps feed the kb-major
        # projection); tiles 1-3: one [128, 8, 512] tile each, single DMA
        xt0 = [pp.tile([128, QT], BF16, tag=f"xt0_{k}", name=f"xt0_{k}") for k in range(N_DIMB)]
        xtc = [
            pp.tile([128, N_DIMB, QT], BF16, tag=f"xtc{c}", name=f"xtc{c}")
            for c in range(1, N_QT)
        ]
        wqk = [pp.tile([128, 512], BF16, tag=f"wqk{k}", name=f"wqk{k}") for k in range(N_DIMB)]
        wv = pp.tile([128, N_DIMB, 256], BF16, tag="wv", name="wv")
        wo = pp.tile([128, 2, DIM], BF16, tag="wo", name="wo")
        # q/k head-dim-major: partition block hp holds heads (2hp, 2hp+1)
        qt = [
            [pp.tile([128, QT], BF16, tag=f"qt{i}_{c}", name=f"qt{i}_{c}") for c in range(N_QT)]
            for i in range(2)
        ]
        kt = [
            [pp.tile([128, QT], BF16, tag=f"kt{i}_{c}", name=f"kt{i}_{c}") for c in range(N_QT)]
            for i in range(2)
        ]
        # v (token-major) + ones columns, per key block: [128, slot, 128]
        # slot order is [h0, h2, h1, h3]; even-slot layout [v | ones],
        # odd-head layout [ones | v] (so AV's denominator lands where the
        # normalize mul wants it).  slot(head h) = (h%2)*2 + h//2.
        vh = [
            pp.tile([128, H_PER_CORE, 2 * HD], BF16, tag=f"vh{b}", name=f"vh{b}")
            for b in range(N_KB)
        ]

        def xt_ap(kb, ct):
            return xt0[kb][:] if ct == 0 else xtc[ct - 1][:, kb, :]

        # ---- warm-up: pin pe_busy_start at ~0 so the 3us p-state ramp is
        # done before real data arrives (Pool memset: runs earliest)
        nc.gpsimd.memset(warm[:], 1.0)
        wps = mmps.tile([128, 512], F32, tag="mm512", name="warm_ps")
        nc.tensor.matmul(wps[:, 0:16], warm[:], warm[:, 0:16], start=True, stop=True)

        # ---- input DMAs (all SP-issued; HWDGE serializes at ~625ns/DMA).
        # Pair order feeds the kb-major tile-0 projection as it lands; wv
        # inserted early (first v matmuls need it ~5us in); the rest after.
        # xt0[0] and wv go through the Pool engine's SWDGE path — a second,
        # parallel DMA issue pipe (HWDGE serializes at ~625ns/DMA)
        nc.gpsimd.dma_start(xt0[0][:], xT[:, 0, 0:QT])
        nc.gpsimd.dma_start(wv[:], wvT[:])
        for kb in range(N_DIMB):
            nc.sync.dma_start(wqk[kb][:], wqkT[:, kb, :])
            if kb > 0:
                nc.sync.dma_start(xt0[kb][:], xT[:, kb, 0:QT])
        # tile-1 columns in two 4-kb chunks (stream into attend(0)'s proj(1)
        # fillers with only 2 HWDGE issue slots)
        for h in range(2):
            nc.sync.dma_start(
                xtc[0][:, h * 4 : (h + 1) * 4, :], xT[:, h * 4 : (h + 1) * 4, QT : 2 * QT]
            )
        for ct in range(2, N_QT):
            cs = slice(ct * QT, (ct + 1) * QT)
            nc.sync.dma_start(xtc[ct - 1][:], xT[:, :, cs])
        nc.sync.dma_start(wo[:], woT[:])

        def vh_fill(tb, src_ps):
            """src_ps: [128, 256] psum = v features for the 4 heads of this
            key block, column order [v_h0 v_h2 v_h1 v_h3]."""
            s4 = src_ps.rearrange("p (s c) -> p s c", s=4)
            nc.vector.memset(vh[tb][:, 0:2, HD : 2 * HD], 1.0)
            nc.vector.memset(vh[tb][:, 2:4, 0:HD], 1.0)
            nc.vector.tensor_copy(vh[tb][:, 0:2, 0:HD], s4[:, 0:2, :])
            nc.vector.tensor_copy(vh[tb][:, 2:4, HD : 2 * HD], s4[:, 2:4, :])

        def proj0():
            """Tile-0 q/k/v projections, kb-major: 8 accumulation chains in
            parallel across 4 psum tiles so each (wqk[kb], xt0[kb]) DMA pair
            is consumed as it arrives."""
            # one accumulation chain per PSUM bank: qk chains in 512-col bank
            # halves of the 2-bank s2 tiles, each v chain in its own 1-bank
            # tile (two concurrent chains in one bank are illegal)
            qk01 = sps.tile([128, 2 * QT], F32, tag="s2", name="p0qk01")
            qk23 = sps.tile([128, 2 * QT], F32, tag="s2", name="p0qk23")
            vps = [
                (mmps if tb < 2 else otps).tile(
                    [128, 256], F32, tag=("mm512" if tb < 2 else "ot"), name=f"p0v{tb}"
                )
                for tb in range(4)
            ]

            def v_mms(kb):
                for tb in range(4):
                    nc.tensor.matmul(
                        vps[tb][:],
                        xt0[kb][:, tb * KB : (tb + 1) * KB],
                        wv[:, kb, :],
                        start=(kb == 0),
                        stop=(kb == N_DIMB - 1),
                    )

            # v matmuls lag the qk stream by 3 kb so the wv DMA (issued 5th)
            # has landed before the first one fires
            for kb in range(N_DIMB):
                for ob in range(4):
                    ps = qk01 if ob < 2 else qk23
                    nc.tensor.matmul(
                        ps[:, (ob % 2) * QT : (ob % 2 + 1) * QT],
                        wqk[kb][:, ob * 128 : (ob + 1) * 128],
                        xt0[kb][:],
                        start=(kb == 0),
                        stop=(kb == N_DIMB - 1),
                    )
                if kb >= 3:
                    v_mms(kb - 3)
            for kb in range(N_DIMB - 3, N_DIMB):
                v_mms(kb)
            nc.vector.tensor_copy(qt[0][0][:], qk01[:, 0:QT])
            nc.vector.tensor_copy(qt[1][0][:], qk01[:, QT : 2 * QT])
            nc.vector.tensor_copy(kt[0][0][:], qk23[:, 0:QT])
            nc.vector.tensor_copy(kt[1][0][:], qk23[:, QT : 2 * QT])
            for tb in range(4):
                vh_fill(tb, vps[tb][:])

        def qk_chain(tt, ob):  # ob 0,1 -> q pair blocks; 2,3 -> k pair blocks
            ps = mmps.tile([128, 512], F32, tag="mm512", name=f"qk_ps{tt}_{ob}")
            for kb in range(N_DIMB):
                nc.tensor.matmul(
                    ps[:],
                    wqk[kb][:, ob * 128 : (ob + 1) * 128],
                    xt_ap(kb, tt),
                    start=(kb == 0),
                    stop=(kb == N_DIMB - 1),
                )
            dest = (qt if ob < 2 else kt)[ob % 2][tt]
            nc.vector.tensor_copy(dest[:], ps[:])

        def v_chain(tb):
            ps = mmps.tile([128, 256], F32, tag="mm512", name=f"v_ps{tb}")
            for kb in range(N_DIMB):
                nc.tensor.matmul(
                    ps[:],
                    xt_ap(kb, tb // 4)[:, (tb % 4) * KB : (tb % 4 + 1) * KB],
                    wv[:, kb, :],
                    start=(kb == 0),
                    stop=(kb == N_DIMB - 1),
                )
            vh_fill(tb, ps[:])

        def proj_pieces(tt):
            for ob in range(4):
                yield lambda ob=ob: qk_chain(tt, ob)
            for tb in range(4 * tt, 4 * tt + 4):
                yield lambda tb=tb: v_chain(tb)

        def attend(tt, os_pair, fillers, tail_fn=None, late=None):
            nb = 4 * (tt + 1)  # allowed key blocks for this query tile
            # pace fillers to finish ~3 steps early so their DVE copies have
            # drained before the final normalize needs the DVE
            n_steps = 2 * nb
            step = 0
            done_fill = 0
            n_fill = len(fillers)

            def fill():
                nonlocal done_fill
                want = (step + 1) * n_fill // n_steps
                while done_fill < want and fillers:
                    fillers.popleft()()
                    done_fill += 1

            for hp in range(2):  # head pair (2hp, 2hp+1)
                ot = [
                    otps.tile([128, QT], F32, tag="ot", name=f"ot{tt}_{hp}_{i}")
                    for i in range(2)
                ]

                def s_mm(b):
                    """S^T for key block b, both heads, into one 2-bank tile."""
                    diag = b - 4 * tt
                    d = diag * 128 if diag >= 0 else 0
                    s = sps.tile([128, 2 * QT], F32, tag="s2", name=f"s{tt}_{hp}_{b}")
                    for i in range(2):
                        rows = slice(i * 64, i * 64 + 64)
                        nc.tensor.matmul(
                            s[:, i * QT + d : (i + 1) * QT],
                            kt[hp][b // 4][rows, (b % 4) * KB : (b % 4 + 1) * KB],
                            qt[hp][tt][rows, d:QT],
                            start=True,
                            stop=True,
                        )
                    p = ppool.tile([128, 2 * QT], BF16, tag="p", name=f"p{tt}_{hp}_{b}")
                    return s, p

                s_tiles = {0: s_mm(0)}
                for b in range(nb):
                    if b + 1 < nb:
                        s_tiles[b + 1] = s_mm(b + 1)
                    diag = b - 4 * tt
                    d = diag * 128 if diag >= 0 else 0
                    s, p = s_tiles.pop(b)
                    if diag < 0:
                        nc.scalar.activation(p[:], s[:], EXP, scale=SCALE)
                    else:
                        # one exp for both heads over cols >= d (all rows),
                        # then zero the masked corner (rows 64-127 of each
                        # head attend only cols >= d+64) AFTER the exp
                        s2 = s[:].rearrange("p (h c) -> p h c", h=2)
                        p2 = p[:].rearrange("p (h c) -> p h c", h=2)
                        nc.scalar.activation(
                            p2[:, :, d:QT], s2[:, :, d:QT], EXP, scale=SCALE
                        )
                        nc.vector.memset(p2[64:128, :, d : d + 64], 0.0)
                    for i in range(2):
                        # slot order [h0, h2, h1, h3]: head 2hp+i -> slot 2i+hp
                        nc.tensor.matmul(
                            ot[i][:, d:QT],
                            vh[b][:, 2 * i + hp, :],
                            p[:, i * QT + d : (i + 1) * QT],
                            start=(b == 0),
                            stop=(b == nb - 1),
                        )
                    fill()
                    step += 1

                # normalize:
                #   ot[0] (even head) = [num | den]; ot[1] (odd) = [den | num]
                #   R[0:64] = 1/den0, R[64:128] = 1/den1 (shifted unary ok)
                #   os[0:64] = ot[0][0:64]*R[0:64]  (aligned)
                #   os[64:128] = ot[1][64:128]*R[64:128]  (aligned)
                if hp == 1 and late:
                    # PE work for the normalize window; their copies go to
                    # the Act engine (idle once the last exp is done) so the
                    # DVE normalize chain is not delayed
                    for piece in late:
                        piece()
                rb = rpool.tile([128, QT], F32, tag="rb", name=f"rb{tt}_{hp}")
                if tail_fn is not None and hp == 1:
                    # per-128-token normalize chunks, each immediately
                    # followed by that token slice's output projection; the
                    # os0-only (db=0) halves of the first two slices are
                    # opened pre-norm so the PE has work during the first
                    # normalize ops
                    tail_open, tail_close = tail_fn
                    yopen = {t4: tail_open(t4) for t4 in range(2)}
                    for t4 in range(4):
                        cs = slice(t4 * 128, (t4 + 1) * 128)
                        nc.vector.reciprocal(rb[0:64, cs], ot[0][64:128, cs])
                        nc.vector.reciprocal(rb[64:128, cs], ot[1][0:64, cs])
                        nc.vector.tensor_mul(
                            os_pair[hp][0:64, cs], ot[0][0:64, cs], rb[0:64, cs]
                        )
                        nc.vector.tensor_mul(
                            os_pair[hp][64:128, cs], ot[1][64:128, cs], rb[64:128, cs]
                        )
                        tail_close(t4, yopen.pop(t4))
                        if t4 + 2 < 4:
                            yopen[t4 + 2] = tail_open(t4 + 2)
                else:
                    # per-head op pairs: ot[0]'s reads finish after two ops,
                    # releasing its psum slot for the next head-pair's AV
                    nc.vector.reciprocal(rb[0:64, :], ot[0][64:128, :])
                    nc.vector.tensor_mul(
                        os_pair[hp][0:64, :], ot[0][0:64, :], rb[0:64, :]
                    )
                    nc.vector.reciprocal(rb[64:128, :], ot[1][0:64, :])
                    nc.vector.tensor_mul(
                        os_pair[hp][64:128, :], ot[1][64:128, :], rb[64:128, :]
                    )

            while fillers:
                fillers.popleft()()

        def y_pieces(tt, os_pair, act_copy_from=99):
            """Output projection for query tile tt, run as attend fillers.
            Copies on DVE (the Act engine is exp-bound inside attends) except
            pieces >= act_copy_from, meant to run after the last exp."""
            pieces = []
            for t4 in range(4):
                trows = slice(t4 * 128, (t4 + 1) * 128)
                ysb = ypool.tile([128, DIM], BF16, tag="ysb", name=f"ysb{tt}_{t4}")
                for jb in range(2):
                    idx = 2 * t4 + jb

                    def piece(t4=t4, jb=jb, ysb=ysb, trows=trows, idx=idx):
                        yps = mmps.tile(
                            [128, 512], F32, tag="mm512", name=f"y_ps{tt}_{t4}_{jb}"
                        )
                        for db in range(2):
                            nc.tensor.matmul(
                                yps[:],
                                os_pair[db][:, trows],
                                wo[:, db, jb * 512 : (jb + 1) * 512],
                                start=(db == 0),
                                stop=(db == 1),
                            )
                        dest = ysb[:, jb * 512 : (jb + 1) * 512]
                        if idx >= act_copy_from:
                            nc.scalar.activation(dest, yps[:], COPY)
                        else:
                            nc.vector.tensor_copy(dest, yps[:])
                        if jb == 1:
                            r0 = tt * QT + t4 * 128
                            nc.sync.dma_start(y[r0 : r0 + 128, :], ysb[:])

                    pieces.append(piece)
            return pieces

        def y_tail_open(tt, os_pair, t4):
            """Start the final tile's output projection for one 128-token
            slice: the db=0 (first head-pair, normalized mid-attend) matmuls
            of both jb halves into one 2-bank s_ps tile.  These only need
            os_pair[0], so they can run while the DVE normalizes os_pair[1]."""
            trows = slice(t4 * 128, (t4 + 1) * 128)
            yps = sps.tile([128, 2 * QT], F32, tag="s2", name=f"yt_ps{tt}_{t4}")
            for jb in range(2):
                nc.tensor.matmul(
                    yps[:, jb * 512 : (jb + 1) * 512],
                    os_pair[0][:, trows],
                    wo[:, 0, jb * 512 : (jb + 1) * 512],
                    start=True,
                    stop=False,
                )
            return yps

        def y_tail_close(tt, os_pair, t4, yps):
            """Finish a tail slice: db=1 accumulation, copy out, DMA."""
            trows = slice(t4 * 128, (t4 + 1) * 128)
            ysb = ypool.tile([128, DIM], BF16, tag="ytb", name=f"yt{tt}_{t4}")
            for jb in range(2):
                nc.tensor.matmul(
                    yps[:, jb * 512 : (jb + 1) * 512],
                    os_pair[1][:, trows],
                    wo[:, 1, jb * 512 : (jb + 1) * 512],
                    start=False,
                    stop=True,
                )
            # full-width Act copy + DMA keeps the DVE free for the normalize
            # chain, which paces this tail; the very last slice splits its
            # copy across DVE+Act halves so the final DMA starts sooner
            r0 = tt * QT + t4 * 128
            if t4 < 3:
                nc.scalar.activation(ysb[:], yps[:], COPY)
                nc.sync.dma_start(y[r0 : r0 + 128, :], ysb[:])
            else:
                # separate half tiles: tile-granular WAW tracking would
                # otherwise serialize the two engines' copies
                y2a = ypool.tile([128, 512], BF16, tag="ytb2a", name=f"yt2a{tt}")
                y2b = ypool.tile([128, 512], BF16, tag="ytb2b", name=f"yt2b{tt}")
                nc.vector.tensor_copy(y2a[:], yps[:, 0:512])
                nc.sync.dma_start(y[r0 : r0 + 128, 0:512], y2a[:])
                nc.scalar.activation(y2b[:], yps[:, 512:1024], COPY)
                nc.sync.dma_start(y[r0 : r0 + 128, 512:1024], y2b[:])

        # ---- the pipeline ----
        # attend(0) <- proj(1); attend(1) <- proj(2)+y(0); attend(2) <-
        # proj(3); attend(3) <- y(1)+y(2) (reserved: the last attend has no
        # next-tile projections to hide its exp latency behind); y(3) tail.
        def interleave(a, b):
            out = deque()
            a, b = deque(a), deque(b)
            while a or b:
                if b:
                    out.append(b.popleft())
                if a:
                    out.append(a.popleft())
            return out

        proj0()
        os_all = []
        for tt in range(N_QT):
            os_all.append(
                [
                    ospool.tile([128, QT], BF16, tag=f"os{i}_{tt}", name=f"os{i}_{tt}")
                    for i in range(2)
                ]
            )

        attend(0, os_all[0], deque(proj_pieces(1)))
        y0 = y_pieces(0, os_all[0])
        attend(1, os_all[1], interleave(proj_pieces(2), y0))
        y1 = y_pieces(1, os_all[1])
        attend(2, os_all[2], deque(proj_pieces(3)))
        y2 = y_pieces(2, os_all[2], act_copy_from=6)
        attend(
            3,
            os_all[3],
            deque(y1 + y2[:6]),
            tail_fn=(
                lambda t4: y_tail_open(3, os_all[3], t4),
                lambda t4, yps: y_tail_close(3, os_all[3], t4, yps),
            ),
            late=y2[6:],
        )


def build():
    global _CACHED_NC
    if _CACHED_NC is not None:
        return _CACHED_NC
    nc = bacc.Bacc(
        "TRN2", target_bir_lowering=False, debug=False, enable_asserts=False
    )
    xT = nc.dram_tensor("xT", [128, N_DIMB, T], BF16, kind="ExternalInput").ap()
    wqkT = nc.dram_tensor("wqkT", [128, N_DIMB, 512], BF16, kind="ExternalInput").ap()
    wvT = nc.dram_tensor("wvT", [128, N_DIMB, 256], BF16, kind="ExternalInput").ap()
    woT = nc.dram_tensor("woutT", [128, 2, DIM], BF16, kind="ExternalInput").ap()
    y = nc.dram_tensor("y", [T, DIM], BF16, kind="ExternalOutput").ap()
    with tile.TileContext(nc) as tc:
        _emit(nc, tc, xT, wqkT, wvT, woT, y)
    nc.compile()
    _CACHED_NC = nc
    return nc


def _to_bf16_3d(mat2d, inner):
    """[R, C] f32 -> [128, R//128, C] bf16 with row index (kb*128+p) -> [p, kb]."""
    import ml_dtypes

    r, c = mat2d.shape
    assert r % 128 == 0 and c == inner
    return np.ascontiguousarray(
        mat2d.reshape(r // 128, 128, c).transpose(1, 0, 2)
    ).astype(ml_dtypes.bfloat16)


def make_in_maps(x, Wqkv, Wout):
    """Host-side sharding: core c = (batch c//4, head-group c%4)."""
    in_maps = []
    for c in range(8):
        b, hg = divmod(c, 4)
        hs = hg * H_PER_CORE
        r0, r1 = hs * HD, (hs + H_PER_CORE) * HD
        qrows = Wqkv[r0:r1]
        krows = Wqkv[DIM + r0 : DIM + r1]
        vrows = Wqkv[2 * DIM + r0 : 2 * DIM + r1]
        # v head blocks reordered [h0, h2, h1, h3] to match the vh slot order
        vperm = np.concatenate(
            [vrows[0:64], vrows[128:192], vrows[64:128], vrows[192:256]], 0
        )
        in_maps.append(
            {
                "xT": _to_bf16_3d(np.ascontiguousarray(x[b].T), T),
                "wqkT": _to_bf16_3d(
                    np.ascontiguousarray(np.concatenate([qrows, krows], 0).T), 512
                ),
                "wvT": _to_bf16_3d(np.ascontiguousarray(vperm.T), 256),
                "woutT": _to_bf16_3d(np.ascontiguousarray(Wout[:, r0:r1].T), DIM),
            }
        )
    return in_maps


def kernel(x, Wqkv, Wout):
    x = np.asarray(x, dtype=np.float32)
    Wqkv = np.asarray(Wqkv, dtype=np.float32)
    Wout = np.asarray(Wout, dtype=np.float32)
    nc = build()
    in_maps = make_in_maps(x, Wqkv, Wout)
    res = run_bass_kernel_spmd(nc, in_maps, core_ids=list(range(8)))
    out = np.zeros((B, T, DIM), np.float32)
    for c in range(8):
        out[c // 4] += res.results[c]["y"].astype(np.float32)
    return out
